# revision 2
# baseline (speedup 1.0000x reference)
"""Differentiable Preisach model on 8 Trainium2 NeuronCores.

Strategy: shard the hysteron/mesh dimension M=20301 across 8 cores
(padded to 8*2560, density=0 padding). Per core, hysterons are laid out
as 20 blocks x 128 partitions with TIME (2048 steps) along the SBUF free
dimension. The sequential relay recurrence

    s_t = where(rising_t, max(s, tanh(500*(h_t - alpha))),
                          min(s, tanh(500*(h_t - beta))))

is expressed as s_t = min(max(s_{t-1}, U_t), D_t) where U_t is the rising
tanh (or -1 on falling steps, neutral for max since s >= -1) and D_t the
falling tanh (or +1 on rising steps, neutral for min since s <= 1); the
per-step direction is known on the host, so it is folded into
per-timestep bias rows (saturating the tanh on inactive steps) while the
per-hysteron thresholds go into the ACT per-partition bias. One DVE
tensor_tensor_scan instruction evaluates the entire 2048-step recurrence
for 128 hysterons at once, and the density-weighted hysteron sum is a PE
matmul (contraction along partitions) accumulated over 20 blocks in PSUM.

Raw Bass (no TileContext): the static pipeline ACT -> DVE scan -> PE
matmul is double-buffered by hand with one semaphore per engine and at
most one wait per instruction (this walrus build rejects multi-wait sync
instructions).
"""

import numpy as np

import concourse.bass as bass
import concourse.mybir as mybir
from concourse.bass_utils import run_bass_kernel_spmd

T = 2048
M = 20301
NCORES = 8
NBLK = 20            # 128-partition hysteron blocks per core
MC = NBLK * 128      # padded hysterons per core (2560)
SCALE = 500.0        # 1 / (2 * temp), temp = 1e-3
BIG = 30000.0        # saturating bias => tanh == -/+1 exactly
F32 = mybir.dt.float32
BF16 = mybir.dt.bfloat16

_prog_cache = {}


def _build_program(state_bf16: bool):
    nc = bass.Bass("TRN2", target_bir_lowering=False, debug=False)
    s_dt = BF16 if state_bf16 else F32

    hup = nc.dram_tensor("hup", [128, T], F32, kind="ExternalInput").ap()
    hdn = nc.dram_tensor("hdn", [128, T], F32, kind="ExternalInput").ap()
    negA = nc.dram_tensor("negA", [128, NBLK], F32, kind="ExternalInput").ap()
    negB = nc.dram_tensor("negB", [128, NBLK], F32, kind="ExternalInput").ap()
    rho = nc.dram_tensor("rho", [128, NBLK], s_dt, kind="ExternalInput").ap()
    outp = nc.dram_tensor("outp", [1, T], F32, kind="ExternalOutput").ap()

    tanh = mybir.ActivationFunctionType.Tanh
    amax = mybir.AluOpType.max
    amin = mybir.AluOpType.min

    with (
        nc.sbuf_tensor([128, T], F32) as hup_t,
        nc.sbuf_tensor([128, T], F32) as hdn_t,
        nc.sbuf_tensor([128, NBLK], F32) as negA_t,
        nc.sbuf_tensor([128, NBLK], F32) as negB_t,
        nc.sbuf_tensor([128, NBLK], s_dt) as rho_t,
        nc.sbuf_tensor([128, T], s_dt) as u0,
        nc.sbuf_tensor([128, T], s_dt) as u1,
        nc.sbuf_tensor([128, T], s_dt) as d0,
        nc.sbuf_tensor([128, T], s_dt) as d1,
        nc.sbuf_tensor([128, T], s_dt) as s0,
        nc.sbuf_tensor([128, T], s_dt) as s1,
        nc.sbuf_tensor([1, T], F32) as out_t,
        nc.psum_tensor([1, T], F32) as ps,
        nc.semaphore("dma_sem") as dma_sem,
        nc.semaphore("act_sem") as act_sem,
        nc.semaphore("dve_sem") as dve_sem,
        nc.semaphore("pe_sem") as pe_sem,
        nc.Block() as block,
    ):
        ubuf = [u0, u1]
        dbuf = [d0, d1]
        sbuf = [s0, s1]

        @block.sync
        def _(sync):
            sync.dma_start(hup_t[:], hup[:]).then_inc(dma_sem, 16)
            sync.dma_start(hdn_t[:], hdn[:]).then_inc(dma_sem, 16)
            sync.dma_start(negA_t[:], negA[:]).then_inc(dma_sem, 16)
            sync.dma_start(negB_t[:], negB[:]).then_inc(dma_sem, 16)
            sync.dma_start(rho_t[:], rho[:]).then_inc(dma_sem, 16)
            sync.wait_ge(dve_sem, NBLK + 4)
            sync.dma_start(outp[:], out_t[:]).then_inc(dma_sem, 16)
            sync.wait_ge(dma_sem, 96)

        @block.scalar
        def _(scalar):
            for g in range(NBLK):
                # u_g = tanh(500*h_t - 500*alpha) (or -1 on falling steps)
                iu = scalar.activation(ubuf[g % 2][:], hup_t[:], tanh,
                                       bias=negA_t[:, g:g + 1])
                if g == 0:
                    iu._wait_ge(dma_sem, 80)
                elif g >= 2:
                    # scan_{g-2} (DVE op g-1) released u/d bufs of g-2
                    iu._wait_ge(dve_sem, g - 1)
                iu.then_inc(act_sem, 1)
                # d_g = tanh(500*h_t - 500*beta) (or +1 on rising steps)
                idn = scalar.activation(dbuf[g % 2][:], hdn_t[:], tanh,
                                        bias=negB_t[:, g:g + 1])
                if g >= 2:
                    # matmuls of g-2 released s buf (carried for scan_g)
                    idn._wait_ge(pe_sem, 4 * (g - 1))
                idn.then_inc(act_sem, 1)

        @block.vector
        def _(vector):
            for g in range(NBLK):
                isc = vector.tensor_tensor_scan(
                    sbuf[g % 2][:], ubuf[g % 2][:], dbuf[g % 2][:],
                    initial=-1.0, op0=amax, op1=amin)
                isc._wait_ge(act_sem, 2 * g + 2)
                isc.then_inc(dve_sem, 1)
            for j in range(4):
                cp = vector.tensor_copy(out_t[:, j * 512:(j + 1) * 512],
                                        ps[:, j * 512:(j + 1) * 512])
                if j == 0:
                    cp._wait_ge(pe_sem, 4 * NBLK)
                cp.then_inc(dve_sem, 1)

        @block.tensor
        def _(tensor):
            for g in range(NBLK):
                for j in range(4):
                    mm = tensor.matmul(
                        ps[:, j * 512:(j + 1) * 512],
                        rho_t[:, g:g + 1],
                        sbuf[g % 2][:, j * 512:(j + 1) * 512],
                        start=(g == 0), stop=(g == NBLK - 1))
                    if j == 0:
                        mm._wait_ge(dve_sem, g + 1)
                    mm.then_inc(pe_sem, 1)

    return nc


def _prepare_in_maps(h, density, mesh, state_bf16: bool):
    hf = np.asarray(h, dtype=np.float32).reshape(-1)
    prev = np.empty_like(hf)
    prev[0] = np.float32(0.0)
    prev[1:] = hf[:-1]
    rising = hf > prev

    bias_up = np.where(rising, SCALE * hf, np.float32(-BIG)).astype(np.float32)
    bias_dn = np.where(rising, np.float32(BIG), SCALE * hf).astype(np.float32)
    hup_rep = np.ascontiguousarray(np.broadcast_to(bias_up, (128, T)))
    hdn_rep = np.ascontiguousarray(np.broadcast_to(bias_dn, (128, T)))

    mesh = np.asarray(mesh, dtype=np.float32)
    density = np.asarray(density, dtype=np.float32)
    Mpad = NCORES * MC
    alpha = np.zeros(Mpad, np.float32)
    alpha[:M] = mesh[:, 1]
    beta = np.zeros(Mpad, np.float32)
    beta[:M] = mesh[:, 0]
    rho = np.zeros(Mpad, np.float32)
    rho[:M] = density

    in_maps = []
    for c in range(NCORES):
        sl = slice(c * MC, (c + 1) * MC)
        negA_c = np.ascontiguousarray((-SCALE * alpha[sl]).reshape(NBLK, 128).T)
        negB_c = np.ascontiguousarray((-SCALE * beta[sl]).reshape(NBLK, 128).T)
        rho_c = np.ascontiguousarray(rho[sl].reshape(NBLK, 128).T)
        if state_bf16:
            import ml_dtypes
            rho_c = rho_c.astype(ml_dtypes.bfloat16)
        in_maps.append({
            "hup": hup_rep, "hdn": hdn_rep,
            "negA": negA_c, "negB": negB_c, "rho": rho_c,
        })
    return in_maps


def _postprocess(results, h, density):
    parts = np.stack([np.asarray(results[c]["outp"], dtype=np.float32).reshape(-1)
                      for c in range(NCORES)])
    m = parts.sum(axis=0) / np.float32(np.asarray(density, np.float32).sum())
    h32 = np.asarray(h, dtype=np.float32).reshape(T, 1)
    return (m.astype(np.float32).reshape(T, 1) + h32).astype(np.float32)


def kernel(h, density, mesh, _state_bf16=False):
    key = bool(_state_bf16)
    if key not in _prog_cache:
        _prog_cache[key] = _build_program(key)
    nc = _prog_cache[key]
    in_maps = _prepare_in_maps(h, density, mesh, key)
    res = run_bass_kernel_spmd(nc, in_maps, core_ids=list(range(NCORES)))
    return _postprocess(res.results, h, density)
